# revision 16
# baseline (speedup 1.0000x reference)
"""Trainium2 Bass kernel for nn_Attention_68298569941449.

out[b,h] = g1*diag(nz_b) + g2*softmax(q_h k_h^T / 64) - g3*outer(nz_b,nz_b)/nnz_b
with q = hs @ Wq.T, k = hs @ Wk.T, nz = (mask == 0);  output [4,16,1024,1024] f32.

Sharding: 64 (batch, head) pairs over 8 NeuronCores -> core c handles batch
c//2 and heads (c%2)*8 .. (c%2)*8+8.  No collectives; host marshals per-core
transposed fp8 operands and concatenates the per-core [8,1024,1024] outputs.

Device schedule per core (all in one Tile graph):
- Projections and scores run on the PE in fp8e4m3 DoubleRow (host passes
  hs.T and 16*W.T as fp8; the 16*16 scale folds into the exp scale 2^-14,
  exact).  Scores carry the q-side fp8 quantization residual in the second
  DoubleRow contraction slot (k's slot is a stride-0 broadcast), restoring
  q to ~fp16 precision for free.  PSUM accumulates in fp32.
- softmax: one ACT pass per [128,1024] tile computes exp(s * 2^-14) AND the
  row sums (accum_out); DVE reciprocal + gamma_2 scale per half-head.
- epilogue: one DVE scalar_tensor_tensor per tile: out = e*inv[row] + A,
  where A = g1*diag(nz) - g3*outer(nz,nz)/nnz is precomputed once per core
  (PE rank-1 outer product + identity diagonal trick).
- software pipeline: proj(pt) on the PE overlaps the exp/epilogue/DMA
  stream of the previous head pair; 512KB contiguous output DMAs.
"""

import numpy as np
from contextlib import ExitStack

import concourse.bass as bass
import concourse.mybir as mybir
import concourse.tile as tile
from concourse import bacc
from concourse.bass_utils import run_bass_kernel_spmd
from concourse.masks import make_identity

B = 4
NT = 1024
DIM = 1024
NH = 16
HD = 64
NHL = 8
QD = NHL * HD
P = 128
KC = DIM // P
RT = NT // P
NPT = QD // P
W_PRESCALE = 16.0
SCALE = 1.0 / (64.0 * W_PRESCALE * W_PRESCALE)

F32 = mybir.dt.float32
BF16 = mybir.dt.bfloat16
FP8 = mybir.dt.float8e4
I32 = mybir.dt.int32
AX = mybir.AxisListType
ALU = mybir.AluOpType
ACTF = mybir.ActivationFunctionType
DR = mybir.MatmulPerfMode.DoubleRow

_CACHE = {}


def _slot_broadcast(ap2d):
    return bass.AP(
        tensor=ap2d.tensor,
        offset=ap2d.offset,
        ap=[ap2d.ap[0], [0, 2], ap2d.ap[1]],
    )


def _build():
    nc = bacc.Bacc()
    hsT = nc.declare_dram_parameter("hsT", [DIM, NT], FP8, isOutput=False)
    wqT = nc.declare_dram_parameter("wqT", [DIM, QD], FP8, isOutput=False)
    wkT = nc.declare_dram_parameter("wkT", [DIM, QD], FP8, isOutput=False)
    mask = nc.declare_dram_parameter("mask", [NT], I32, isOutput=False)
    g = nc.declare_dram_parameter("g", [1, 3], F32, isOutput=False)
    out = nc.declare_dram_parameter("out", [NHL, NT, NT], F32, isOutput=True)

    with tile.TileContext(nc) as tc, ExitStack() as ctx:
        singles = ctx.enter_context(tc.tile_pool(name="singles", bufs=1))
        ppool = ctx.enter_context(tc.tile_pool(name="ps", bufs=4, space="PSUM"))
        epool = ctx.enter_context(tc.tile_pool(name="e", bufs=5))
        opool = ctx.enter_context(tc.tile_pool(name="o", bufs=6))
        small = ctx.enter_context(tc.tile_pool(name="small", bufs=4))

        m_pc = singles.tile([P, RT], I32)
        nc.gpsimd.dma_start(out=m_pc, in_=mask[:].rearrange("(a p) -> p a", p=P))
        m_row = singles.tile([1, NT], I32)
        nc.gpsimd.dma_start(out=m_row, in_=mask[:].rearrange("(a n) -> a n", a=1))
        g_row = singles.tile([1, 3], F32)
        nc.gpsimd.dma_start(out=g_row, in_=g[:])
        gap = g[:]
        g1b = singles.tile([P, 1], F32)
        g2b = singles.tile([P, 1], F32)
        nc.gpsimd.dma_start(
            out=g1b, in_=bass.AP(tensor=gap.tensor, offset=0, ap=[[0, P], [1, 1]])
        )
        nc.gpsimd.dma_start(
            out=g2b, in_=bass.AP(tensor=gap.tensor, offset=1, ap=[[0, P], [1, 1]])
        )

        sb_hsT = singles.tile([P, KC, NT], FP8)
        nc.sync.dma_start(
            out=sb_hsT, in_=hsT[:, :].rearrange("(kc p) t -> p kc t", p=P)
        )
        sb_wqT = singles.tile([P, KC, QD], FP8)
        nc.sync.dma_start(
            out=sb_wqT, in_=wqT[:, :].rearrange("(kc p) q -> p kc q", p=P)
        )
        sb_wkT = singles.tile([P, KC, QD], FP8)
        nc.sync.dma_start(
            out=sb_wkT, in_=wkT[:, :].rearrange("(kc p) q -> p kc q", p=P)
        )

        ident = singles.tile([P, P], F32)
        make_identity(nc, ident)

        nz_col = singles.tile([P, RT], F32)
        nc.vector.tensor_scalar(nz_col, m_pc, 0, None, ALU.is_equal)
        nz_colg1 = singles.tile([P, RT], F32)
        nc.vector.tensor_scalar(nz_colg1, nz_col, g1b, None, ALU.mult)
        nz_row = singles.tile([1, NT], BF16)
        nc.vector.tensor_scalar(nz_row, m_row, 0, None, ALU.is_equal)

        nnz = small.tile([1, 1], F32)
        nc.vector.tensor_reduce(nnz, nz_row, axis=AX.X, op=ALU.add)
        inv_nnz = small.tile([1, 1], F32)
        nc.vector.reciprocal(inv_nnz, nnz)
        u_scale = small.tile([1, 1], F32)
        nc.vector.tensor_scalar(
            u_scale, inv_nnz, g_row[0:1, 2:3], -1.0, ALU.mult, ALU.mult
        )
        u_row = singles.tile([1, NT], BF16)
        nc.vector.tensor_scalar(u_row, nz_row, u_scale, None, ALU.mult)

        sb_A = singles.tile([P, RT, NT], F32)
        for rt in range(RT):
            psA = ppool.tile([P, NT], F32, tag="ps")
            for hf in range(2):
                nc.tensor.matmul(
                    psA[:, hf * 512:(hf + 1) * 512],
                    lhsT=u_row[0:1, rt * P:(rt + 1) * P],
                    rhs=nz_row[0:1, hf * 512:(hf + 1) * 512],
                    start=True,
                    stop=True,
                )
            nc.scalar.copy(out=sb_A[:, rt, :], in_=psA)
            nc.vector.scalar_tensor_tensor(
                out=sb_A[:, rt, rt * P:(rt + 1) * P],
                in0=ident,
                scalar=nz_colg1[:, rt:rt + 1],
                in1=psA[:, rt * P:(rt + 1) * P],
                op0=ALU.mult,
                op1=ALU.add,
            )

        sb_qT = singles.tile([P, NPT, 2, NT], FP8)
        sb_kT = singles.tile([P, NPT, NT], FP8)

        def proj(pt):
            for w_sb, is_q in ((sb_wqT, True), (sb_wkT, False)):
                ps = ppool.tile([P, NT], F32, tag="ps")
                for hf in range(2):
                    for j in range(KC // 2):
                        nc.tensor.matmul(
                            ps[:, hf * 512:(hf + 1) * 512],
                            lhsT=w_sb[:, 2 * j:2 * j + 2, pt * P:(pt + 1) * P],
                            rhs=sb_hsT[:, 2 * j:2 * j + 2,
                                       hf * 512:(hf + 1) * 512],
                            start=(j == 0),
                            stop=(j == KC // 2 - 1),
                            perf_mode=DR,
                        )
                if is_q:
                    nc.scalar.copy(out=sb_qT[:, pt, 0, :], in_=ps)
                    nc.vector.tensor_sub(
                        sb_qT[:, pt, 1, :], ps, sb_qT[:, pt, 0, :]
                    )
                else:
                    nc.vector.tensor_copy(out=sb_kT[:, pt, :], in_=ps)

        def head_stream(h):
            pt, po = h // 2, (h % 2) * HD
            for half in range(2):
                sums = small.tile([P, 4], F32, tag="sums")
                es = []
                for rtl in range(4):
                    rt = half * 4 + rtl
                    psS = ppool.tile([P, NT], F32, tag="ps")
                    for hf in range(2):
                        nc.tensor.matmul(
                            psS[:, hf * 512:(hf + 1) * 512],
                            lhsT=sb_qT[po:po + HD, pt, :, rt * P:(rt + 1) * P],
                            rhs=_slot_broadcast(
                                sb_kT[po:po + HD, pt,
                                      hf * 512:(hf + 1) * 512]
                            ),
                            start=True,
                            stop=True,
                            perf_mode=DR,
                        )
                    e = epool.tile([P, NT], F32, tag="e")
                    nc.scalar.activation(
                        out=e,
                        in_=psS,
                        func=ACTF.Exp,
                        scale=SCALE,
                        accum_out=sums[:, rtl:rtl + 1],
                    )
                    es.append(e)
                inv = small.tile([P, 4], F32, tag="inv")
                nc.vector.reciprocal(inv, sums)
                inv2 = small.tile([P, 4], F32, tag="inv2")
                nc.vector.tensor_scalar(inv2, inv, g2b, None, ALU.mult)
                for rtl in range(4):
                    rt = half * 4 + rtl
                    o = opool.tile([P, NT], F32, tag="o")
                    nc.vector.scalar_tensor_tensor(
                        out=o,
                        in0=es[rtl],
                        scalar=inv2[:, rtl:rtl + 1],
                        in1=sb_A[:, rt, :],
                        op0=ALU.mult,
                        op1=ALU.add,
                    )
                    nc.sync.dma_start(out=out[h, rt * P:(rt + 1) * P, :], in_=o)

        proj(0)
        for pt in range(1, NPT):
            proj(pt)
            head_stream(2 * (pt - 1))
            head_stream(2 * (pt - 1) + 1)
        head_stream(2 * (NPT - 1))
        head_stream(2 * (NPT - 1) + 1)

    nc.compile()
    return nc


def _get_nc():
    if "nc" not in _CACHE:
        _CACHE["nc"] = _build()
    return _CACHE["nc"]


def kernel(hidden_states, attention_mask, Wq, Wk, gamma_1, gamma_2, gamma_3,
           _trace=False):
    hs = np.asarray(hidden_states, dtype=np.float32)
    am = np.asarray(attention_mask, dtype=np.int32)
    Wq = np.asarray(Wq, dtype=np.float32)
    Wk = np.asarray(Wk, dtype=np.float32)
    g = np.array(
        [[float(gamma_1), float(gamma_2), float(gamma_3)]], dtype=np.float32
    )

    nc = _get_nc()
    fp8 = mybir.dt.np(FP8)
    in_maps = []
    for c in range(8):
        b, hg = c // 2, c % 2
        wq = (W_PRESCALE * Wq[hg * QD:(hg + 1) * QD, :]).T
        wk = (W_PRESCALE * Wk[hg * QD:(hg + 1) * QD, :]).T
        in_maps.append(
            {
                "hsT": np.ascontiguousarray(hs[b].T).astype(fp8),
                "wqT": np.ascontiguousarray(wq).astype(fp8),
                "wkT": np.ascontiguousarray(wk).astype(fp8),
                "mask": np.ascontiguousarray(am[b]),
                "g": g,
            }
        )
    res = run_bass_kernel_spmd(nc, in_maps, core_ids=list(range(8)), trace=_trace)
    out = np.empty((B, NH, NT, NT), np.float32)
    for c in range(8):
        b, hg = c // 2, c % 2
        out[b, hg * NHL:(hg + 1) * NHL] = res.results[c]["out"]
    if _trace:
        return out, res
    return out


# revision 17
# speedup vs baseline: 1.0480x; 1.0480x over previous
"""Trainium2 Bass kernel for nn_Attention_68298569941449.

out[b,h] = g1*diag(nz_b) + g2*softmax(q_h k_h^T / 64) - g3*outer(nz_b,nz_b)/nnz_b
with q = hs @ Wq.T, k = hs @ Wk.T, nz = (mask == 0);  output [4,16,1024,1024] f32.

Sharding: 64 (batch, head) pairs over 8 NeuronCores -> core c handles batch
c//2 and heads (c%2)*8 .. (c%2)*8+8.  No collectives; host marshals per-core
transposed fp8 operands and concatenates the per-core [8,1024,1024] outputs.

Device schedule per core (all in one Tile graph):
- Projections and scores run on the PE in fp8e4m3 DoubleRow (host passes
  hs.T and 16*W.T as fp8; the 16*16 scale folds into the exp scale 2^-14,
  exact).  Scores carry the q-side fp8 quantization residual in the second
  DoubleRow contraction slot (k's slot is a stride-0 broadcast), restoring
  q to ~fp16 precision for free.  PSUM accumulates in fp32.
- softmax: one ACT pass per [128,1024] tile computes exp(s * 2^-14) AND the
  row sums (accum_out); DVE reciprocal + gamma_2 scale per half-head.
- epilogue: one DVE scalar_tensor_tensor per tile: out = e*inv[row] + A,
  where A = g1*diag(nz) - g3*outer(nz,nz)/nnz is precomputed once per core
  (PE rank-1 outer product + identity diagonal trick).
- software pipeline: proj(pt) on the PE overlaps the exp/epilogue/DMA
  stream of the previous head pair; 512KB contiguous output DMAs.
"""

import numpy as np
from contextlib import ExitStack

import concourse.bass as bass
import concourse.mybir as mybir
import concourse.tile as tile
from concourse import bacc
from concourse.bass_utils import run_bass_kernel_spmd
from concourse.masks import make_identity

B = 4
NT = 1024
DIM = 1024
NH = 16
HD = 64
NHL = 8
QD = NHL * HD
P = 128
KC = DIM // P
RT = NT // P
NPT = QD // P
W_PRESCALE = 16.0
SCALE = 1.0 / (64.0 * W_PRESCALE * W_PRESCALE)

F32 = mybir.dt.float32
BF16 = mybir.dt.bfloat16
FP8 = mybir.dt.float8e4
I32 = mybir.dt.int32
AX = mybir.AxisListType
ALU = mybir.AluOpType
ACTF = mybir.ActivationFunctionType
DR = mybir.MatmulPerfMode.DoubleRow

_CACHE = {}


def _slot_broadcast(ap2d):
    return bass.AP(
        tensor=ap2d.tensor,
        offset=ap2d.offset,
        ap=[ap2d.ap[0], [0, 2], ap2d.ap[1]],
    )


def _build():
    nc = bacc.Bacc()
    hsT = nc.declare_dram_parameter("hsT", [P, KC, NT], FP8, isOutput=False)
    wqT = nc.declare_dram_parameter("wqT", [P, KC, QD], FP8, isOutput=False)
    wkT = nc.declare_dram_parameter("wkT", [P, KC, QD], FP8, isOutput=False)
    mask = nc.declare_dram_parameter("mask", [NT], I32, isOutput=False)
    g = nc.declare_dram_parameter("g", [1, 3], F32, isOutput=False)
    out = nc.declare_dram_parameter("out", [NHL, NT, NT], F32, isOutput=True)

    with tile.TileContext(nc) as tc, ExitStack() as ctx:
        singles = ctx.enter_context(tc.tile_pool(name="singles", bufs=1))
        ppool = ctx.enter_context(tc.tile_pool(name="ps", bufs=4, space="PSUM"))
        epool = ctx.enter_context(tc.tile_pool(name="e", bufs=5))
        opool = ctx.enter_context(tc.tile_pool(name="o", bufs=6))
        small = ctx.enter_context(tc.tile_pool(name="small", bufs=4))

        m_pc = singles.tile([P, RT], I32)
        nc.sync.dma_start(out=m_pc, in_=mask[:].rearrange("(a p) -> p a", p=P))
        m_row = singles.tile([1, NT], I32)
        nc.sync.dma_start(out=m_row, in_=mask[:].rearrange("(a n) -> a n", a=1))
        g_row = singles.tile([1, 3], F32)
        nc.sync.dma_start(out=g_row, in_=g[:])
        gap = g[:]
        g1b = singles.tile([P, 1], F32)
        g2b = singles.tile([P, 1], F32)
        nc.gpsimd.dma_start(
            out=g1b, in_=bass.AP(tensor=gap.tensor, offset=0, ap=[[0, P], [1, 1]])
        )
        nc.gpsimd.dma_start(
            out=g2b, in_=bass.AP(tensor=gap.tensor, offset=1, ap=[[0, P], [1, 1]])
        )

        sb_hsT = singles.tile([P, KC, NT], FP8)
        sb_wqT = singles.tile([P, KC, QD], FP8)
        sb_wkT = singles.tile([P, KC, QD], FP8)
        nc.sync.dma_start(out=sb_wqT, in_=wqT[:, :, :])
        nc.sync.dma_start(out=sb_hsT[:, 0:4, :], in_=hsT[:, 0:4, :])
        nc.sync.dma_start(out=sb_hsT[:, 4:8, :], in_=hsT[:, 4:8, :])
        nc.sync.dma_start(out=sb_wkT, in_=wkT[:, :, :])

        ident = singles.tile([P, P], F32)
        make_identity(nc, ident)

        nz_col = singles.tile([P, RT], F32)
        nc.vector.tensor_scalar(nz_col, m_pc, 0, None, ALU.is_equal)
        nz_colg1 = singles.tile([P, RT], F32)
        nc.vector.tensor_scalar(nz_colg1, nz_col, g1b, None, ALU.mult)
        nz_row = singles.tile([1, NT], FP8)   # exact 0/1 values
        nc.vector.tensor_scalar(nz_row, m_row, 0, None, ALU.is_equal)

        ones_col = singles.tile([P, 1], F32)
        nc.vector.memset(ones_col, 1.0)
        ps_nnz = ppool.tile([1, RT], F32, tag="ps")
        nc.tensor.matmul(ps_nnz, lhsT=ones_col, rhs=nz_col, start=True, stop=True)
        nnz = small.tile([1, 1], F32)
        nc.vector.tensor_reduce(nnz, ps_nnz, axis=AX.X, op=ALU.add)
        inv_nnz = small.tile([1, 1], F32)
        nc.vector.reciprocal(inv_nnz, nnz)
        u_scale = small.tile([1, 1], F32)  # -256 * g3 / nnz
        nc.vector.tensor_scalar(
            u_scale, inv_nnz, g_row[0:1, 2:3], -256.0, ALU.mult, ALU.mult
        )
        # u2: fp8 DR slot pair (value, residual) -> rank-1 outer at DR speed
        u2 = singles.tile([1, 2, NT], FP8)
        nc.vector.tensor_scalar(u2[0:1, 0, :], nz_row, u_scale, None, ALU.mult)
        nc.vector.scalar_tensor_tensor(
            out=u2[0:1, 1, :],
            in0=nz_row,
            scalar=u_scale,
            in1=u2[0:1, 0, :],
            op0=ALU.mult,
            op1=ALU.subtract,
        )

        sb_A = singles.tile([P, RT, NT], F32)
        for rt in range(RT):
            psA = ppool.tile([P, NT], F32, tag="ps")
            for hf in range(2):
                nc.tensor.matmul(
                    psA[:, hf * 512:(hf + 1) * 512],
                    lhsT=u2[0:1, :, rt * P:(rt + 1) * P],
                    rhs=_slot_broadcast(nz_row[0:1, hf * 512:(hf + 1) * 512]),
                    start=True,
                    stop=True,
                    perf_mode=DR,
                )
            if rt % 2 == 0:
                nc.vector.tensor_scalar(
                    sb_A[:, rt, :], psA, 1.0 / 256.0, None, ALU.mult
                )
            else:
                nc.scalar.mul(out=sb_A[:, rt, :], in_=psA, mul=1.0 / 256.0)
            # diagonal block: (psA/256) + ident * (g1*nz[p])
            idg = small.tile([P, P], F32, tag="idg")
            nc.vector.tensor_scalar(
                idg, ident, nz_colg1[:, rt:rt + 1], None, ALU.mult
            )
            nc.vector.scalar_tensor_tensor(
                out=sb_A[:, rt, rt * P:(rt + 1) * P],
                in0=psA[:, rt * P:(rt + 1) * P],
                scalar=1.0 / 256.0,
                in1=idg,
                op0=ALU.mult,
                op1=ALU.add,
            )

        sb_qT = singles.tile([P, NPT, 2, NT], FP8)
        sb_kT = singles.tile([P, NPT, NT], FP8)

        def proj(pt):
            for w_sb, is_q in ((sb_wqT, True), (sb_wkT, False)):
                ps = ppool.tile([P, NT], F32, tag="ps")
                for hf in range(2):
                    for j in range(KC // 2):
                        nc.tensor.matmul(
                            ps[:, hf * 512:(hf + 1) * 512],
                            lhsT=w_sb[:, 2 * j:2 * j + 2, pt * P:(pt + 1) * P],
                            rhs=sb_hsT[:, 2 * j:2 * j + 2,
                                       hf * 512:(hf + 1) * 512],
                            start=(j == 0),
                            stop=(j == KC // 2 - 1),
                            perf_mode=DR,
                        )
                if is_q:
                    nc.scalar.copy(out=sb_qT[:, pt, 0, :], in_=ps)
                    nc.vector.tensor_sub(
                        sb_qT[:, pt, 1, :], ps, sb_qT[:, pt, 0, :]
                    )
                else:
                    nc.vector.tensor_copy(out=sb_kT[:, pt, :], in_=ps)

        def head_stream(h):
            pt, po = h // 2, (h % 2) * HD
            for half in range(2):
                sums = small.tile([P, 4], F32, tag="sums")
                es = []
                for rtl in range(4):
                    rt = half * 4 + rtl
                    psS = ppool.tile([P, NT], F32, tag="ps")
                    for hf in range(2):
                        nc.tensor.matmul(
                            psS[:, hf * 512:(hf + 1) * 512],
                            lhsT=sb_qT[po:po + HD, pt, :, rt * P:(rt + 1) * P],
                            rhs=_slot_broadcast(
                                sb_kT[po:po + HD, pt,
                                      hf * 512:(hf + 1) * 512]
                            ),
                            start=True,
                            stop=True,
                            perf_mode=DR,
                        )
                    e = epool.tile([P, NT], F32, tag="e")
                    nc.scalar.activation(
                        out=e,
                        in_=psS,
                        func=ACTF.Exp,
                        scale=SCALE,
                        accum_out=sums[:, rtl:rtl + 1],
                    )
                    es.append(e)
                inv = small.tile([P, 4], F32, tag="inv")
                nc.vector.reciprocal(inv, sums)
                inv2 = small.tile([P, 4], F32, tag="inv2")
                nc.vector.tensor_scalar(inv2, inv, g2b, None, ALU.mult)
                for rtl in range(4):
                    rt = half * 4 + rtl
                    o = opool.tile([P, NT], F32, tag="o")
                    nc.vector.scalar_tensor_tensor(
                        out=o,
                        in0=es[rtl],
                        scalar=inv2[:, rtl:rtl + 1],
                        in1=sb_A[:, rt, :],
                        op0=ALU.mult,
                        op1=ALU.add,
                    )
                    nc.sync.dma_start(out=out[h, rt * P:(rt + 1) * P, :], in_=o)

        proj(0)
        for pt in range(1, NPT):
            proj(pt)
            head_stream(2 * (pt - 1))
            head_stream(2 * (pt - 1) + 1)
        head_stream(2 * (NPT - 1))
        head_stream(2 * (NPT - 1) + 1)

    nc.compile()
    return nc


def _get_nc():
    if "nc" not in _CACHE:
        _CACHE["nc"] = _build()
    return _CACHE["nc"]


def kernel(hidden_states, attention_mask, Wq, Wk, gamma_1, gamma_2, gamma_3,
           _trace=False):
    hs = np.asarray(hidden_states, dtype=np.float32)
    am = np.asarray(attention_mask, dtype=np.int32)
    Wq = np.asarray(Wq, dtype=np.float32)
    Wk = np.asarray(Wk, dtype=np.float32)
    g = np.array(
        [[float(gamma_1), float(gamma_2), float(gamma_3)]], dtype=np.float32
    )

    nc = _get_nc()
    fp8 = mybir.dt.np(FP8)
    in_maps = []
    for c in range(8):
        b, hg = c // 2, c % 2
        wq = (W_PRESCALE * Wq[hg * QD:(hg + 1) * QD, :]).T
        wk = (W_PRESCALE * Wk[hg * QD:(hg + 1) * QD, :]).T

        def chunk(a):   # [DIM, x] -> [P, KC, x], partition-major contiguous
            return np.ascontiguousarray(
                a.reshape(KC, P, a.shape[1]).transpose(1, 0, 2)
            )

        in_maps.append(
            {
                "hsT": chunk(hs[b].T.astype(fp8)),
                "wqT": chunk(wq.astype(fp8)),
                "wkT": chunk(wk.astype(fp8)),
                "mask": np.ascontiguousarray(am[b]),
                "g": g,
            }
        )
    res = run_bass_kernel_spmd(nc, in_maps, core_ids=list(range(8)), trace=_trace)
    out = np.empty((B, NH, NT, NT), np.float32)
    for c in range(8):
        b, hg = c // 2, c % 2
        out[b, hg * NHL:(hg + 1) * NHL] = res.results[c]["out"]
    if _trace:
        return out, res
    return out
